# revision 2
# baseline (speedup 1.0000x reference)
"""DabDetrAttention kernel for Trainium2, data-parallel over batch on 8 cores.

Strategy (per core, 4 batches):
  - Host-side prep: K^T / hidden^T (d-major) layouts, V chunks augmented with a
    ones column (softmax denominator rides along MM2 for free), W_out^T, bias
    broadcast. All matmuls run as fp32r (TF32-class, ~1.5e-4 rel err).
  - Per (batch, head): scores^T [kv, q] via K=64 matmuls row-packed two heads
    per PE pass; exp on ScalarE straight out of PSUM (3 score slots per
    instruction, scale=1/8 folded in); attn^T accumulated over 16 kv-chunks
    with M=33 matmuls (32 v-dims + ones row = denominator); normalize via
    reciprocal + gpsimd partition-broadcast + VectorE multiply; output
    projection as two accumulating matmuls per 128-row q-tile.
"""

import numpy as np

import concourse.bacc as bacc
import concourse.tile as tile
from concourse import mybir
from concourse.bass_utils import run_bass_kernel_spmd

N_CORES = 8
B, Q, KV, D, H = 32, 300, 2048, 512, 8
DH, VD, O = 64, 32, 256
BPC = B // N_CORES          # batches per core
NC_KV = KV // 128           # kv chunks (16)
SLOT_W = 512                # psum bank width in fp32
EXP_SLOTS = 3               # score slots per exp instruction
QT = [(0, 128), (128, 128), (256, 44)]  # q tiles for the output projection

F32 = mybir.dt.float32
F32R = mybir.dt.float32r
Exp = mybir.ActivationFunctionType.Exp

_cached_nc = None


def _build():
    nc = bacc.Bacc("TRN2", target_bir_lowering=False, debug=False)
    KT = nc.dram_tensor("KT", [BPC, D, KV], F32, kind="ExternalInput").ap()
    HT = nc.dram_tensor("HT", [BPC, D, Q], F32, kind="ExternalInput").ap()
    VP = nc.dram_tensor("VP", [BPC, NC_KV, 128, H * 33], F32, kind="ExternalInput").ap()
    WT = nc.dram_tensor("WT", [O, O], F32, kind="ExternalInput").ap()
    BB = nc.dram_tensor("BB", [128, O], F32, kind="ExternalInput").ap()
    OUT = nc.dram_tensor("OUT", [BPC, Q, O], F32, kind="ExternalOutput").ap()

    with tile.TileContext(nc) as tc:
        with (
            tc.tile_pool(name="consts", bufs=1) as consts,
            tc.tile_pool(name="ktp", bufs=2) as ktp,
            tc.tile_pool(name="htp", bufs=2) as htp,
            tc.tile_pool(name="vpp", bufs=2) as vpp,
            tc.tile_pool(name="etp", bufs=4) as etp,
            tc.tile_pool(name="attp", bufs=2) as attp,
            tc.tile_pool(name="smalls", bufs=8) as smalls,
            tc.tile_pool(name="outp", bufs=3) as outp,
            tc.tile_pool(name="eps", bufs=2, space="PSUM") as eps,
            tc.tile_pool(name="aps", bufs=2, space="PSUM") as aps,
        ):
            wt = consts.tile([128, 2, O], F32R, tag="wt")
            nc.sync.dma_start(
                out=wt[:], in_=WT.rearrange("(t p) o -> p t o", p=128).bitcast(F32R)
            )
            bb = consts.tile([128, O], F32, tag="bb")
            nc.sync.dma_start(out=bb[:], in_=BB)

            for b in range(BPC):
                kt = ktp.tile([128, 4, KV], F32R, tag="kt")
                nc.sync.dma_start(
                    out=kt[:],
                    in_=KT[b].rearrange("(t p) k -> p t k", p=128).bitcast(F32R),
                )
                ht = htp.tile([128, 4, Q], F32R, tag="ht")
                nc.sync.dma_start(
                    out=ht[:],
                    in_=HT[b].rearrange("(t p) q -> p t q", p=128).bitcast(F32R),
                )
                vp = vpp.tile([128, NC_KV, H * 33], F32R, tag="vp")
                nc.sync.dma_start(
                    out=vp[:], in_=VP[b].rearrange("c p w -> p c w").bitcast(F32R)
                )
                attnT = attp.tile([128, 2, Q], F32R, tag="attnT")

                # flat slot stream: (pair t, chunk c, head-half h2)
                slots = [
                    (t, c, h2)
                    for t in range(4)
                    for c in range(NC_KV)
                    for h2 in range(2)
                ]
                accs = {}
                done = 0
                while done < len(slots):
                    group = slots[done : done + EXP_SLOTS]
                    n = len(group)
                    exp_ps = eps.tile([128, EXP_SLOTS * SLOT_W], F32, tag="exp")
                    for s, (t, c, h2) in enumerate(group):
                        lo = 64 * h2
                        nc.tensor.matmul(
                            exp_ps[:, s * SLOT_W : s * SLOT_W + Q].bitcast(F32),
                            kt[lo : lo + 64, t, c * 128 : (c + 1) * 128],
                            ht[lo : lo + 64, t, :],
                            start=True,
                            stop=True,
                            tile_position=(lo, 0),
                        )
                    et = etp.tile([128, EXP_SLOTS, Q], F32R, tag="et")
                    nc.scalar.activation(
                        out=et[:, 0:n, :],
                        in_=exp_ps[:]
                        .rearrange("p (s w) -> p s w", s=EXP_SLOTS)[:, 0:n, 0:Q],
                        func=Exp,
                        scale=float(DH) ** -0.5,
                    )
                    for s, (t, c, h2) in enumerate(group):
                        h = 2 * t + h2
                        if (t, h2) not in accs:
                            accs[(t, h2)] = aps.tile([33, Q], F32, tag="acc", name=f"acc_{b}_{t}_{h2}")
                        nc.tensor.matmul(
                            accs[(t, h2)][0:33, :],
                            vp[:, c, h * 33 : h * 33 + 33],
                            et[:, s, :],
                            start=(c == 0),
                            stop=(c == NC_KV - 1),
                        )
                    done += n
                    # finalize any pair whose last slot was in this group
                    for t, c, h2 in group:
                        if c == NC_KV - 1 and h2 == 1:
                            for hh in range(2):
                                acc = accs.pop((t, hh))
                                h = 2 * t + hh
                                recip = smalls.tile([1, Q], F32, tag="recip")
                                nc.vector.reciprocal(
                                    out=recip[:], in_=acc[32:33, :]
                                )
                                rb = smalls.tile([32, Q], F32, tag="rb")
                                nc.gpsimd.partition_broadcast(rb[:], recip[:])
                                nc.vector.tensor_mul(
                                    out=attnT[
                                        (h % 4) * 32 : (h % 4) * 32 + 32, h // 4, :
                                    ],
                                    in0=acc[0:32, :],
                                    in1=rb[:],
                                )

                # output projection: OUT[b, q, :] = attnT.T @ WT + bias
                for q0, w in QT:
                    ps3 = aps.tile([128, SLOT_W], F32, tag="acc")
                    for ci in range(2):
                        nc.tensor.matmul(
                            ps3[0:w, 0:O],
                            attnT[:, ci, q0 : q0 + w],
                            wt[:, ci, :],
                            start=(ci == 0),
                            stop=(ci == 1),
                        )
                    out_sb = outp.tile([128, O], F32, tag="out")
                    nc.vector.tensor_add(
                        out=out_sb[0:w, :], in0=ps3[0:w, 0:O], in1=bb[0:w, :]
                    )
                    nc.sync.dma_start(
                        out=OUT[b, q0 : q0 + w, :], in_=out_sb[0:w, :]
                    )

    nc.compile()
    return nc


def _prep_core_inputs(hidden_states, key_states, value_states, W_out, b_out, c):
    lo, hi = c * BPC, (c + 1) * BPC
    hs = hidden_states[lo:hi]
    ks = key_states[lo:hi]
    vs = value_states[lo:hi]
    kt = np.ascontiguousarray(ks.transpose(0, 2, 1), dtype=np.float32)
    ht = np.ascontiguousarray(hs.transpose(0, 2, 1), dtype=np.float32)
    v4 = vs.reshape(BPC, NC_KV, 128, H, VD)
    vp = np.empty((BPC, NC_KV, 128, H, 33), dtype=np.float32)
    vp[..., :VD] = v4
    vp[..., VD] = 1.0
    vp = vp.reshape(BPC, NC_KV, 128, H * 33)
    wt = np.ascontiguousarray(W_out.T, dtype=np.float32)
    bbt = np.broadcast_to(
        np.asarray(b_out, dtype=np.float32)[None, :], (128, O)
    ).copy()
    return {"KT": kt, "HT": ht, "VP": vp, "WT": wt, "BB": bbt}


def kernel(hidden_states, key_states, value_states, W_out, b_out):
    global _cached_nc
    hidden_states = np.asarray(hidden_states, dtype=np.float32)
    key_states = np.asarray(key_states, dtype=np.float32)
    value_states = np.asarray(value_states, dtype=np.float32)
    W_out = np.asarray(W_out, dtype=np.float32)
    b_out = np.asarray(b_out, dtype=np.float32)

    in_maps = [
        _prep_core_inputs(hidden_states, key_states, value_states, W_out, b_out, c)
        for c in range(N_CORES)
    ]
    if _cached_nc is None:
        _cached_nc = _build()
    res = run_bass_kernel_spmd(_cached_nc, in_maps, list(range(N_CORES)))
    return np.concatenate([r["OUT"] for r in res.results], axis=0)


# revision 4
# speedup vs baseline: 1.2224x; 1.2224x over previous
"""DabDetrAttention kernel for Trainium2, data-parallel over batch on 8 cores.

Strategy (per core, 4 batches):
  - Host-side prep: K^T / hidden^T (d-major) layouts, V chunks augmented with a
    ones column (softmax denominator rides along MM2 for free), W_out^T, bias
    broadcast. All matmuls run as fp32r (TF32-class, ~1.5e-4 rel err).
  - Per (batch, head): scores^T [kv, q] via K=64 matmuls row-packed two heads
    per PE pass; exp on ScalarE straight out of PSUM (3 score slots per
    instruction, scale=1/8 folded in); attn^T accumulated over 16 kv-chunks
    with M=33 matmuls (32 v-dims + ones row = denominator); normalize via
    reciprocal + gpsimd partition-broadcast + VectorE multiply; output
    projection as two accumulating matmuls per 128-row q-tile.
"""

import ml_dtypes
import numpy as np

import concourse.bacc as bacc
import concourse.tile as tile
from concourse import mybir
from concourse.bass_utils import run_bass_kernel_spmd

N_CORES = 8
B, Q, KV, D, H = 32, 300, 2048, 512, 8
DH, VD, O = 64, 32, 256
BPC = B // N_CORES          # batches per core
NC_KV = KV // 128           # kv chunks (16)
SLOT_W = 512                # psum bank width in fp32
EXP_SLOTS = 3               # score slots per exp instruction
QT = [(0, 128), (128, 128), (256, 44)]  # q tiles for the output projection

F32 = mybir.dt.float32
F32R = mybir.dt.float32r
BF16 = mybir.dt.bfloat16
NPBF16 = ml_dtypes.bfloat16
Exp = mybir.ActivationFunctionType.Exp

_cached_nc = None


def _build():
    nc = bacc.Bacc("TRN2", target_bir_lowering=False, debug=False)
    KT = nc.dram_tensor("KT", [BPC, D, KV], BF16, kind="ExternalInput").ap()
    HT = nc.dram_tensor("HT", [BPC, D, Q], BF16, kind="ExternalInput").ap()
    VP = nc.dram_tensor("VP", [BPC, NC_KV, 128, H * 33], BF16, kind="ExternalInput").ap()
    WT = nc.dram_tensor("WT", [O, O], BF16, kind="ExternalInput").ap()
    BB = nc.dram_tensor("BB", [128, O], F32, kind="ExternalInput").ap()
    OUT = nc.dram_tensor("OUT", [BPC, Q, O], F32, kind="ExternalOutput").ap()

    with tile.TileContext(nc) as tc:
        with (
            tc.tile_pool(name="consts", bufs=1) as consts,
            tc.tile_pool(name="ktp", bufs=2) as ktp,
            tc.tile_pool(name="htp", bufs=2) as htp,
            tc.tile_pool(name="vpp", bufs=2) as vpp,
            tc.tile_pool(name="etp", bufs=4) as etp,
            tc.tile_pool(name="attp", bufs=2) as attp,
            tc.tile_pool(name="smalls", bufs=8) as smalls,
            tc.tile_pool(name="outp", bufs=3) as outp,
            tc.tile_pool(name="eps", bufs=2, space="PSUM") as eps,
            tc.tile_pool(name="aps", bufs=2, space="PSUM") as aps,
        ):
            wt = consts.tile([128, 2, O], BF16, tag="wt")
            nc.sync.dma_start(
                out=wt[:], in_=WT.rearrange("(t p) o -> p t o", p=128)
            )
            bb = consts.tile([128, O], F32, tag="bb")
            nc.sync.dma_start(out=bb[:], in_=BB)

            for b in range(BPC):
                kt = ktp.tile([128, 4, KV], BF16, tag="kt")
                nc.sync.dma_start(
                    out=kt[:],
                    in_=KT[b].rearrange("(t p) k -> p t k", p=128),
                )
                ht = htp.tile([128, 4, Q], BF16, tag="ht")
                nc.sync.dma_start(
                    out=ht[:],
                    in_=HT[b].rearrange("(t p) q -> p t q", p=128),
                )
                vp = vpp.tile([128, NC_KV, H * 33], BF16, tag="vp")
                nc.sync.dma_start(
                    out=vp[:], in_=VP[b].rearrange("c p w -> p c w")
                )
                attnT = attp.tile([128, 2, Q], BF16, tag="attnT")

                # flat slot stream: (pair t, chunk c, head-half h2)
                slots = [
                    (t, c, h2)
                    for t in range(4)
                    for c in range(NC_KV)
                    for h2 in range(2)
                ]
                accs = {}
                done = 0
                while done < len(slots):
                    group = slots[done : done + EXP_SLOTS]
                    n = len(group)
                    exp_ps = eps.tile([128, EXP_SLOTS * SLOT_W], F32, tag="exp")
                    for s, (t, c, h2) in enumerate(group):
                        lo = 64 * h2
                        nc.tensor.matmul(
                            exp_ps[:, s * SLOT_W : s * SLOT_W + Q].bitcast(F32),
                            kt[lo : lo + 64, t, c * 128 : (c + 1) * 128],
                            ht[lo : lo + 64, t, :],
                            start=True,
                            stop=True,
                            tile_position=(lo, 0),
                        )
                    et = etp.tile([128, EXP_SLOTS, Q], BF16, tag="et")
                    nc.scalar.activation(
                        out=et[:, 0:n, :],
                        in_=exp_ps[:]
                        .rearrange("p (s w) -> p s w", s=EXP_SLOTS)[:, 0:n, 0:Q],
                        func=Exp,
                        scale=float(DH) ** -0.5,
                    )
                    for s, (t, c, h2) in enumerate(group):
                        h = 2 * t + h2
                        if (t, h2) not in accs:
                            accs[(t, h2)] = aps.tile([33, Q], F32, tag="acc", name=f"acc_{b}_{t}_{h2}")
                        nc.tensor.matmul(
                            accs[(t, h2)][0:33, :],
                            vp[:, c, h * 33 : h * 33 + 33],
                            et[:, s, :],
                            start=(c == 0),
                            stop=(c == NC_KV - 1),
                        )
                    done += n
                    # finalize any pair whose last slot was in this group
                    for t, c, h2 in group:
                        if c == NC_KV - 1 and h2 == 1:
                            for hh in range(2):
                                acc = accs.pop((t, hh))
                                h = 2 * t + hh
                                recip = smalls.tile([1, Q], F32, tag="recip")
                                nc.vector.reciprocal(
                                    out=recip[:], in_=acc[32:33, :]
                                )
                                rb = smalls.tile([32, Q], F32, tag="rb")
                                nc.gpsimd.partition_broadcast(rb[:], recip[:])
                                nc.vector.tensor_mul(
                                    out=attnT[
                                        (h % 4) * 32 : (h % 4) * 32 + 32, h // 4, :
                                    ],
                                    in0=acc[0:32, :],
                                    in1=rb[:],
                                )

                # output projection: OUT[b, q, :] = attnT.T @ WT + bias
                for q0, w in QT:
                    ps3 = aps.tile([128, SLOT_W], F32, tag="acc")
                    for ci in range(2):
                        nc.tensor.matmul(
                            ps3[0:w, 0:O],
                            attnT[:, ci, q0 : q0 + w],
                            wt[:, ci, :],
                            start=(ci == 0),
                            stop=(ci == 1),
                        )
                    out_sb = outp.tile([128, O], F32, tag="out")
                    nc.vector.tensor_add(
                        out=out_sb[0:w, :], in0=ps3[0:w, 0:O], in1=bb[0:w, :]
                    )
                    nc.sync.dma_start(
                        out=OUT[b, q0 : q0 + w, :], in_=out_sb[0:w, :]
                    )

    nc.compile()
    return nc


def _prep_core_inputs(hidden_states, key_states, value_states, W_out, b_out, c):
    lo, hi = c * BPC, (c + 1) * BPC
    hs = hidden_states[lo:hi]
    ks = key_states[lo:hi]
    vs = value_states[lo:hi]
    kt = np.ascontiguousarray(ks.transpose(0, 2, 1)).astype(NPBF16)
    ht = np.ascontiguousarray(hs.transpose(0, 2, 1)).astype(NPBF16)
    v4 = vs.reshape(BPC, NC_KV, 128, H, VD)
    vp = np.empty((BPC, NC_KV, 128, H, 33), dtype=NPBF16)
    vp[..., :VD] = v4
    vp[..., VD] = 1.0
    vp = vp.reshape(BPC, NC_KV, 128, H * 33)
    wt = np.ascontiguousarray(W_out.T).astype(NPBF16)
    bbt = np.broadcast_to(
        np.asarray(b_out, dtype=np.float32)[None, :], (128, O)
    ).copy()
    return {"KT": kt, "HT": ht, "VP": vp, "WT": wt, "BB": bbt}


def kernel(hidden_states, key_states, value_states, W_out, b_out):
    global _cached_nc
    hidden_states = np.asarray(hidden_states, dtype=np.float32)
    key_states = np.asarray(key_states, dtype=np.float32)
    value_states = np.asarray(value_states, dtype=np.float32)
    W_out = np.asarray(W_out, dtype=np.float32)
    b_out = np.asarray(b_out, dtype=np.float32)

    in_maps = [
        _prep_core_inputs(hidden_states, key_states, value_states, W_out, b_out, c)
        for c in range(N_CORES)
    ]
    if _cached_nc is None:
        _cached_nc = _build()
    res = run_bass_kernel_spmd(_cached_nc, in_maps, list(range(N_CORES)))
    return np.concatenate([r["OUT"] for r in res.results], axis=0)


# revision 6
# speedup vs baseline: 1.4999x; 1.2271x over previous
"""DabDetrAttention kernel for Trainium2, data-parallel over batch on 8 cores.

Strategy (per core, 4 batches):
  - Host-side prep: K^T / hidden^T (d-major) layouts, V chunks augmented with a
    ones column (softmax denominator rides along MM2 for free), W_out^T, bias
    broadcast. All matmuls run as fp32r (TF32-class, ~1.5e-4 rel err).
  - Per (batch, head): scores^T [kv, q] via K=64 matmuls row-packed two heads
    per PE pass; exp on ScalarE straight out of PSUM (3 score slots per
    instruction, scale=1/8 folded in); attn^T accumulated over 16 kv-chunks
    with M=33 matmuls (32 v-dims + ones row = denominator); normalize via
    reciprocal + gpsimd partition-broadcast + VectorE multiply; output
    projection as two accumulating matmuls per 128-row q-tile.
"""

import ml_dtypes
import numpy as np

import concourse.bacc as bacc
import concourse.tile as tile
from concourse import mybir
from concourse.bass_utils import run_bass_kernel_spmd

N_CORES = 8
B, Q, KV, D, H = 32, 300, 2048, 512, 8
DH, VD, O = 64, 32, 256
BPC = B // N_CORES          # batches per core
NC_KV = KV // 128           # kv chunks (16)
SLOT_W = 512                # psum bank width in fp32
EXP_SLOTS = 3               # score slots per exp instruction
QT = [(0, 128), (128, 128), (256, 44)]  # q tiles for the output projection

F32 = mybir.dt.float32
F32R = mybir.dt.float32r
BF16 = mybir.dt.bfloat16
NPBF16 = ml_dtypes.bfloat16
Exp = mybir.ActivationFunctionType.Exp

_cached_nc = None


def _build():
    nc = bacc.Bacc("TRN2", target_bir_lowering=False, debug=False)
    KT = nc.dram_tensor("KT", [BPC, D, KV], BF16, kind="ExternalInput").ap()
    HT = nc.dram_tensor("HT", [BPC, D, Q], BF16, kind="ExternalInput").ap()
    VP = nc.dram_tensor("VP", [BPC, NC_KV, 128, H * 33], BF16, kind="ExternalInput").ap()
    WT = nc.dram_tensor("WT", [O, O], BF16, kind="ExternalInput").ap()
    BB = nc.dram_tensor("BB", [128, O], F32, kind="ExternalInput").ap()
    OUT = nc.dram_tensor("OUT", [BPC, Q, O], F32, kind="ExternalOutput").ap()

    with tile.TileContext(nc) as tc:
        with (
            tc.tile_pool(name="consts", bufs=1) as consts,
            tc.tile_pool(name="ktp", bufs=2) as ktp,
            tc.tile_pool(name="htp", bufs=2) as htp,
            tc.tile_pool(name="vpp", bufs=2) as vpp,
            tc.tile_pool(name="etp", bufs=4) as etp,
            tc.tile_pool(name="attp", bufs=2) as attp,
            tc.tile_pool(name="smalls", bufs=8) as smalls,
            tc.tile_pool(name="outp", bufs=3) as outp,
            tc.tile_pool(name="eps", bufs=2, space="PSUM") as eps,
            tc.tile_pool(name="aps", bufs=2, space="PSUM") as aps,
        ):
            wt = consts.tile([128, 2, O], BF16, tag="wt")
            nc.sync.dma_start(
                out=wt[:], in_=WT.rearrange("(t p) o -> p t o", p=128)
            )
            bb = consts.tile([128, O], F32, tag="bb")
            nc.sync.dma_start(out=bb[:], in_=BB)

            for b in range(BPC):
                kt = ktp.tile([128, 4, KV], BF16, tag="kt")
                nc.sync.dma_start(
                    out=kt[:],
                    in_=KT[b].rearrange("(t p) k -> p t k", p=128),
                )
                ht = htp.tile([128, 4, Q], BF16, tag="ht")
                nc.sync.dma_start(
                    out=ht[:],
                    in_=HT[b].rearrange("(t p) q -> p t q", p=128),
                )
                vp = vpp.tile([128, NC_KV, H * 33], BF16, tag="vp")
                nc.sync.dma_start(
                    out=vp[:], in_=VP[b].rearrange("c p w -> p c w")
                )
                attnT = attp.tile([128, 2, Q], BF16, tag="attnT")

                # flat slot stream: (pair t, chunk c, head-half h2)
                slots = [
                    (t, c, h2)
                    for t in range(4)
                    for c in range(NC_KV)
                    for h2 in range(2)
                ]
                accs = {}
                done = 0
                while done < len(slots):
                    group = slots[done : done + EXP_SLOTS]
                    n = len(group)
                    exp_ps = eps.tile([128, EXP_SLOTS * SLOT_W], F32, tag="exp")
                    for s, (t, c, h2) in enumerate(group):
                        lo = 64 * h2
                        nc.tensor.matmul(
                            exp_ps[:, s * SLOT_W : s * SLOT_W + Q].bitcast(F32),
                            kt[lo : lo + 64, t, c * 128 : (c + 1) * 128],
                            ht[lo : lo + 64, t, :],
                            start=True,
                            stop=True,
                            tile_position=(lo, 0),
                        )
                    et = etp.tile([128, EXP_SLOTS, Q], BF16, tag="et")
                    nc.scalar.activation(
                        out=et[:, 0:n, :],
                        in_=exp_ps[:]
                        .rearrange("p (s w) -> p s w", s=EXP_SLOTS)[:, 0:n, 0:Q],
                        func=Exp,
                        scale=float(DH) ** -0.5,
                    )
                    for s, (t, c, h2) in enumerate(group):
                        h = 2 * t + h2
                        if (t, h2) not in accs:
                            accs[(t, h2)] = aps.tile([33, Q], F32, tag="acc", name=f"acc_{b}_{t}_{h2}")
                        nc.tensor.matmul(
                            accs[(t, h2)][0:33, :],
                            vp[:, c, h * 33 : h * 33 + 33],
                            et[:, s, :],
                            start=(c == 0),
                            stop=(c == NC_KV - 1),
                        )
                    done += n
                    # finalize any pair whose last slot was in this group
                    for t, c, h2 in group:
                        if c == NC_KV - 1 and h2 == 1:
                            for hh in range(2):
                                acc = accs.pop((t, hh))
                                h = 2 * t + hh
                                # copy to SBUF right away to free the psum bank
                                stage = smalls.tile([33, Q], F32, tag="stage")
                                nc.vector.tensor_copy(out=stage[:], in_=acc[:])
                                recip = smalls.tile([1, Q], F32, tag="recip")
                                nc.vector.reciprocal(
                                    out=recip[:], in_=stage[32:33, :]
                                )
                                rb = smalls.tile([32, Q], F32, tag="rb")
                                nc.gpsimd.partition_broadcast(rb[:], recip[:])
                                nc.vector.tensor_mul(
                                    out=attnT[
                                        (h % 4) * 32 : (h % 4) * 32 + 32, h // 4, :
                                    ],
                                    in0=stage[0:32, :],
                                    in1=rb[:],
                                )

                # output projection: OUT[b, q, :] = attnT.T @ WT + bias
                for q0, w in QT:
                    ps3 = aps.tile([128, SLOT_W], F32, tag="acc")
                    for ci in range(2):
                        nc.tensor.matmul(
                            ps3[0:w, 0:O],
                            attnT[:, ci, q0 : q0 + w],
                            wt[:, ci, :],
                            start=(ci == 0),
                            stop=(ci == 1),
                        )
                    out_sb = outp.tile([128, O], F32, tag="out")
                    nc.vector.tensor_add(
                        out=out_sb[0:w, :], in0=ps3[0:w, 0:O], in1=bb[0:w, :]
                    )
                    nc.sync.dma_start(
                        out=OUT[b, q0 : q0 + w, :], in_=out_sb[0:w, :]
                    )

    nc.compile()
    return nc


def _prep_core_inputs(hidden_states, key_states, value_states, W_out, b_out, c):
    lo, hi = c * BPC, (c + 1) * BPC
    hs = hidden_states[lo:hi]
    ks = key_states[lo:hi]
    vs = value_states[lo:hi]
    kt = np.ascontiguousarray(ks.transpose(0, 2, 1)).astype(NPBF16)
    ht = np.ascontiguousarray(hs.transpose(0, 2, 1)).astype(NPBF16)
    v4 = vs.reshape(BPC, NC_KV, 128, H, VD)
    vp = np.empty((BPC, NC_KV, 128, H, 33), dtype=NPBF16)
    vp[..., :VD] = v4
    vp[..., VD] = 1.0
    vp = vp.reshape(BPC, NC_KV, 128, H * 33)
    wt = np.ascontiguousarray(W_out.T).astype(NPBF16)
    bbt = np.broadcast_to(
        np.asarray(b_out, dtype=np.float32)[None, :], (128, O)
    ).copy()
    return {"KT": kt, "HT": ht, "VP": vp, "WT": wt, "BB": bbt}


def kernel(hidden_states, key_states, value_states, W_out, b_out):
    global _cached_nc
    hidden_states = np.asarray(hidden_states, dtype=np.float32)
    key_states = np.asarray(key_states, dtype=np.float32)
    value_states = np.asarray(value_states, dtype=np.float32)
    W_out = np.asarray(W_out, dtype=np.float32)
    b_out = np.asarray(b_out, dtype=np.float32)

    in_maps = [
        _prep_core_inputs(hidden_states, key_states, value_states, W_out, b_out, c)
        for c in range(N_CORES)
    ]
    if _cached_nc is None:
        _cached_nc = _build()
    res = run_bass_kernel_spmd(_cached_nc, in_maps, list(range(N_CORES)))
    return np.concatenate([r["OUT"] for r in res.results], axis=0)


# revision 7
# speedup vs baseline: 1.5128x; 1.0086x over previous
"""DabDetrAttention kernel for Trainium2, data-parallel over batch on 8 cores.

Strategy (per core, 4 batches):
  - Host-side prep: K^T / hidden^T (d-major) layouts, V chunks augmented with a
    ones column (softmax denominator rides along MM2 for free), W_out^T, bias
    broadcast. All matmuls run as fp32r (TF32-class, ~1.5e-4 rel err).
  - Per (batch, head): scores^T [kv, q] via K=64 matmuls row-packed two heads
    per PE pass; exp on ScalarE straight out of PSUM (3 score slots per
    instruction, scale=1/8 folded in); attn^T accumulated over 16 kv-chunks
    with M=33 matmuls (32 v-dims + ones row = denominator); normalize via
    reciprocal + gpsimd partition-broadcast + VectorE multiply; output
    projection as two accumulating matmuls per 128-row q-tile.
"""

import ml_dtypes
import numpy as np

import concourse.bacc as bacc
import concourse.tile as tile
from concourse import mybir
from concourse.bass_utils import run_bass_kernel_spmd

N_CORES = 8
B, Q, KV, D, H = 32, 300, 2048, 512, 8
DH, VD, O = 64, 32, 256
BPC = B // N_CORES          # batches per core
NC_KV = KV // 128           # kv chunks (16)
SLOT_W = 512                # psum bank width in fp32
EXP_SLOTS = 3               # score slots per exp instruction
QT = [(0, 128), (128, 128), (256, 44)]  # q tiles for the output projection

F32 = mybir.dt.float32
F32R = mybir.dt.float32r
BF16 = mybir.dt.bfloat16
NPBF16 = ml_dtypes.bfloat16
Exp = mybir.ActivationFunctionType.Exp

_cached_nc = None


def _build():
    nc = bacc.Bacc("TRN2", target_bir_lowering=False, debug=False)
    KT = nc.dram_tensor("KT", [BPC, D, KV], BF16, kind="ExternalInput").ap()
    HT = nc.dram_tensor("HT", [BPC, D, Q], BF16, kind="ExternalInput").ap()
    VP = nc.dram_tensor("VP", [BPC, NC_KV, 128, H * 33], BF16, kind="ExternalInput").ap()
    WT = nc.dram_tensor("WT", [O, O], BF16, kind="ExternalInput").ap()
    BB = nc.dram_tensor("BB", [128, O], F32, kind="ExternalInput").ap()
    OUT = nc.dram_tensor("OUT", [BPC, Q, O], F32, kind="ExternalOutput").ap()

    with tile.TileContext(nc) as tc:
        with (
            tc.tile_pool(name="consts", bufs=1) as consts,
            tc.tile_pool(name="ktp", bufs=2) as ktp,
            tc.tile_pool(name="htp", bufs=2) as htp,
            tc.tile_pool(name="vpp", bufs=2) as vpp,
            tc.tile_pool(name="etp", bufs=4) as etp,
            tc.tile_pool(name="attp", bufs=2) as attp,
            tc.tile_pool(name="smalls", bufs=8) as smalls,
            tc.tile_pool(name="outp", bufs=3) as outp,
            tc.tile_pool(name="eps", bufs=2, space="PSUM") as eps,
            tc.tile_pool(name="aps", bufs=2, space="PSUM") as aps,
        ):
            wt = consts.tile([128, 2, O], BF16, tag="wt")
            nc.sync.dma_start(
                out=wt[:], in_=WT.rearrange("(t p) o -> p t o", p=128)
            )
            bb = consts.tile([128, O], F32, tag="bb")
            nc.sync.dma_start(out=bb[:], in_=BB)

            def emit_loads(b):
                kt = ktp.tile([128, 4, KV], BF16, tag="kt", name=f"kt{b}")
                for t in range(4):
                    nc.sync.dma_start(
                        out=kt[:, t, :],
                        in_=KT[b].rearrange("(t p) k -> p t k", p=128)[:, t, :],
                    )
                ht = htp.tile([128, 4, Q], BF16, tag="ht", name=f"ht{b}")
                nc.sync.dma_start(
                    out=ht[:],
                    in_=HT[b].rearrange("(t p) q -> p t q", p=128),
                )
                vp = vpp.tile([128, NC_KV, H * 33], BF16, tag="vp", name=f"vp{b}")
                for half in range(2):
                    cs = half * (NC_KV // 2)
                    ce = cs + NC_KV // 2
                    nc.sync.dma_start(
                        out=vp[:, cs:ce, :],
                        in_=VP[b, cs:ce].rearrange("c p w -> p c w"),
                    )
                return kt, ht, vp

            loaded = {0: emit_loads(0)}
            for b in range(BPC):
                kt, ht, vp = loaded.pop(b)
                if b + 1 < BPC:
                    loaded[b + 1] = emit_loads(b + 1)
                attnT = attp.tile([128, 2, Q], BF16, tag="attnT")

                # flat slot stream: (pair t, chunk c, head-half h2)
                slots = [
                    (t, c, h2)
                    for t in range(4)
                    for c in range(NC_KV)
                    for h2 in range(2)
                ]
                accs = {}
                done = 0
                while done < len(slots):
                    group = slots[done : done + EXP_SLOTS]
                    n = len(group)
                    exp_ps = eps.tile([128, EXP_SLOTS * SLOT_W], F32, tag="exp")
                    for s, (t, c, h2) in enumerate(group):
                        lo = 64 * h2
                        nc.tensor.matmul(
                            exp_ps[:, s * SLOT_W : s * SLOT_W + Q].bitcast(F32),
                            kt[lo : lo + 64, t, c * 128 : (c + 1) * 128],
                            ht[lo : lo + 64, t, :],
                            start=True,
                            stop=True,
                            tile_position=(lo, 0),
                        )
                    et = etp.tile([128, EXP_SLOTS, Q], BF16, tag="et")
                    nc.scalar.activation(
                        out=et[:, 0:n, :],
                        in_=exp_ps[:]
                        .rearrange("p (s w) -> p s w", s=EXP_SLOTS)[:, 0:n, 0:Q],
                        func=Exp,
                        scale=float(DH) ** -0.5,
                    )
                    for s, (t, c, h2) in enumerate(group):
                        h = 2 * t + h2
                        if (t, h2) not in accs:
                            accs[(t, h2)] = aps.tile([33, Q], F32, tag="acc", name=f"acc_{b}_{t}_{h2}")
                        nc.tensor.matmul(
                            accs[(t, h2)][0:33, :],
                            vp[:, c, h * 33 : h * 33 + 33],
                            et[:, s, :],
                            start=(c == 0),
                            stop=(c == NC_KV - 1),
                        )
                    done += n
                    # finalize any pair whose last slot was in this group
                    for t, c, h2 in group:
                        if c == NC_KV - 1 and h2 == 1:
                            for hh in range(2):
                                acc = accs.pop((t, hh))
                                h = 2 * t + hh
                                # copy to SBUF right away to free the psum bank
                                stage = smalls.tile([33, Q], F32, tag="stage")
                                nc.vector.tensor_copy(out=stage[:], in_=acc[:])
                                recip = smalls.tile([1, Q], F32, tag="recip")
                                nc.vector.reciprocal(
                                    out=recip[:], in_=stage[32:33, :]
                                )
                                rb = smalls.tile([32, Q], F32, tag="rb")
                                nc.gpsimd.partition_broadcast(rb[:], recip[:])
                                nc.vector.tensor_mul(
                                    out=attnT[
                                        (h % 4) * 32 : (h % 4) * 32 + 32, h // 4, :
                                    ],
                                    in0=stage[0:32, :],
                                    in1=rb[:],
                                )

                # output projection: OUT[b, q, :] = attnT.T @ WT + bias
                for q0, w in QT:
                    ps3 = aps.tile([128, SLOT_W], F32, tag="acc")
                    for ci in range(2):
                        nc.tensor.matmul(
                            ps3[0:w, 0:O],
                            attnT[:, ci, q0 : q0 + w],
                            wt[:, ci, :],
                            start=(ci == 0),
                            stop=(ci == 1),
                        )
                    out_sb = outp.tile([128, O], F32, tag="out")
                    nc.vector.tensor_add(
                        out=out_sb[0:w, :], in0=ps3[0:w, 0:O], in1=bb[0:w, :]
                    )
                    nc.gpsimd.dma_start(
                        out=OUT[b, q0 : q0 + w, :], in_=out_sb[0:w, :]
                    )

    nc.compile()
    return nc


def _prep_core_inputs(hidden_states, key_states, value_states, W_out, b_out, c):
    lo, hi = c * BPC, (c + 1) * BPC
    hs = hidden_states[lo:hi]
    ks = key_states[lo:hi]
    vs = value_states[lo:hi]
    kt = np.ascontiguousarray(ks.transpose(0, 2, 1)).astype(NPBF16)
    ht = np.ascontiguousarray(hs.transpose(0, 2, 1)).astype(NPBF16)
    v4 = vs.reshape(BPC, NC_KV, 128, H, VD)
    vp = np.empty((BPC, NC_KV, 128, H, 33), dtype=NPBF16)
    vp[..., :VD] = v4
    vp[..., VD] = 1.0
    vp = vp.reshape(BPC, NC_KV, 128, H * 33)
    wt = np.ascontiguousarray(W_out.T).astype(NPBF16)
    bbt = np.broadcast_to(
        np.asarray(b_out, dtype=np.float32)[None, :], (128, O)
    ).copy()
    return {"KT": kt, "HT": ht, "VP": vp, "WT": wt, "BB": bbt}


def kernel(hidden_states, key_states, value_states, W_out, b_out):
    global _cached_nc
    hidden_states = np.asarray(hidden_states, dtype=np.float32)
    key_states = np.asarray(key_states, dtype=np.float32)
    value_states = np.asarray(value_states, dtype=np.float32)
    W_out = np.asarray(W_out, dtype=np.float32)
    b_out = np.asarray(b_out, dtype=np.float32)

    in_maps = [
        _prep_core_inputs(hidden_states, key_states, value_states, W_out, b_out, c)
        for c in range(N_CORES)
    ]
    if _cached_nc is None:
        _cached_nc = _build()
    res = run_bass_kernel_spmd(_cached_nc, in_maps, list(range(N_CORES)))
    return np.concatenate([r["OUT"] for r in res.results], axis=0)


# revision 8
# speedup vs baseline: 1.5348x; 1.0145x over previous
"""DabDetrAttention kernel for Trainium2, data-parallel over batch on 8 cores.

Strategy (per core, 4 batches):
  - Host-side prep: K^T / hidden^T (d-major) layouts, V chunks augmented with a
    ones column (softmax denominator rides along MM2 for free), W_out^T, bias
    broadcast. All matmuls run as fp32r (TF32-class, ~1.5e-4 rel err).
  - Per (batch, head): scores^T [kv, q] via K=64 matmuls row-packed two heads
    per PE pass; exp on ScalarE straight out of PSUM (3 score slots per
    instruction, scale=1/8 folded in); attn^T accumulated over 16 kv-chunks
    with M=33 matmuls (32 v-dims + ones row = denominator); normalize via
    reciprocal + gpsimd partition-broadcast + VectorE multiply; output
    projection as two accumulating matmuls per 128-row q-tile.
"""

import ml_dtypes
import numpy as np

import concourse.bacc as bacc
import concourse.tile as tile
from concourse import mybir
from concourse.bass_utils import run_bass_kernel_spmd

N_CORES = 8
B, Q, KV, D, H = 32, 300, 2048, 512, 8
DH, VD, O = 64, 32, 256
BPC = B // N_CORES          # batches per core
NC_KV = KV // 128           # kv chunks (16)
SLOT_W = 512                # psum bank width in fp32
EXP_SLOTS = 3               # score slots per exp instruction
QT = [(0, 128), (128, 128), (256, 44)]  # q tiles for the output projection

F32 = mybir.dt.float32
F32R = mybir.dt.float32r
BF16 = mybir.dt.bfloat16
NPBF16 = ml_dtypes.bfloat16
Exp = mybir.ActivationFunctionType.Exp

_cached_nc = None


def _build():
    nc = bacc.Bacc("TRN2", target_bir_lowering=False, debug=False)
    KT = nc.dram_tensor("KT", [BPC, D, KV], BF16, kind="ExternalInput").ap()
    HT = nc.dram_tensor("HT", [BPC, D, Q], BF16, kind="ExternalInput").ap()
    VP = nc.dram_tensor("VP", [BPC, NC_KV, 128, H * 33], BF16, kind="ExternalInput").ap()
    WT = nc.dram_tensor("WT", [O, O], BF16, kind="ExternalInput").ap()
    BB = nc.dram_tensor("BB", [128, O], F32, kind="ExternalInput").ap()
    OUT = nc.dram_tensor("OUT", [BPC, Q, O], F32, kind="ExternalOutput").ap()

    with tile.TileContext(nc) as tc:
        with (
            tc.tile_pool(name="consts", bufs=1) as consts,
            tc.tile_pool(name="ktp", bufs=2) as ktp,
            tc.tile_pool(name="htp", bufs=2) as htp,
            tc.tile_pool(name="vpp", bufs=2) as vpp,
            tc.tile_pool(name="etp", bufs=4) as etp,
            tc.tile_pool(name="attp", bufs=2) as attp,
            tc.tile_pool(name="smalls", bufs=8) as smalls,
            tc.tile_pool(name="outp", bufs=3) as outp,
            tc.tile_pool(name="eps", bufs=2, space="PSUM") as eps,
            tc.tile_pool(name="aps", bufs=2, space="PSUM") as aps,
        ):
            wt = consts.tile([128, 2, O], BF16, tag="wt")
            nc.sync.dma_start(
                out=wt[:], in_=WT.rearrange("(t p) o -> p t o", p=128)
            )
            bb = consts.tile([128, O], F32, tag="bb")
            nc.sync.dma_start(out=bb[:], in_=BB)

            def emit_loads(b):
                kt = ktp.tile([128, 4, KV], BF16, tag="kt", name=f"kt{b}")
                for t in range(4):
                    nc.sync.dma_start(
                        out=kt[:, t, :],
                        in_=KT[b].rearrange("(t p) k -> p t k", p=128)[:, t, :],
                    )
                ht = htp.tile([128, 4, Q], BF16, tag="ht", name=f"ht{b}")
                nc.sync.dma_start(
                    out=ht[:],
                    in_=HT[b].rearrange("(t p) q -> p t q", p=128),
                )
                vp = vpp.tile([128, NC_KV, H * 33], BF16, tag="vp", name=f"vp{b}")
                for half in range(2):
                    cs = half * (NC_KV // 2)
                    ce = cs + NC_KV // 2
                    nc.sync.dma_start(
                        out=vp[:, cs:ce, :],
                        in_=VP[b, cs:ce].rearrange("c p w -> p c w"),
                    )
                return kt, ht, vp

            loaded = {0: emit_loads(0)}
            pending_mm3 = []

            def emit_mm3(b, attnT):
                for q0, w in QT:
                    ps3 = aps.tile([128, SLOT_W], F32, tag="acc", name=f"mm3_{b}_{q0}")
                    for ci in range(2):
                        nc.tensor.matmul(
                            ps3[0:w, 0:O],
                            attnT[:, ci, q0 : q0 + w],
                            wt[:, ci, :],
                            start=(ci == 0),
                            stop=(ci == 1),
                        )
                    out_sb = outp.tile([128, O], F32, tag="out", name=f"o_{b}_{q0}")
                    nc.vector.tensor_add(
                        out=out_sb[0:w, :], in0=ps3[0:w, 0:O], in1=bb[0:w, :]
                    )
                    nc.gpsimd.dma_start(
                        out=OUT[b, q0 : q0 + w, :], in_=out_sb[0:w, :]
                    )

            for b in range(BPC):
                kt, ht, vp = loaded.pop(b)
                if b + 1 < BPC:
                    loaded[b + 1] = emit_loads(b + 1)
                attnT = attp.tile([128, 2, Q], BF16, tag="attnT")

                # flat slot stream: (pair t, chunk c, head-half h2)
                slots = [
                    (t, c, h2)
                    for t in range(4)
                    for c in range(NC_KV)
                    for h2 in range(2)
                ]
                accs = {}
                done = 0
                while done < len(slots):
                    group = slots[done : done + EXP_SLOTS]
                    n = len(group)
                    exp_ps = eps.tile([128, EXP_SLOTS * SLOT_W], F32, tag="exp")
                    for s, (t, c, h2) in enumerate(group):
                        lo = 64 * h2
                        nc.tensor.matmul(
                            exp_ps[:, s * SLOT_W : s * SLOT_W + Q].bitcast(F32),
                            kt[lo : lo + 64, t, c * 128 : (c + 1) * 128],
                            ht[lo : lo + 64, t, :],
                            start=True,
                            stop=True,
                            tile_position=(lo, 0),
                        )
                    et = etp.tile([128, EXP_SLOTS, Q], BF16, tag="et")
                    nc.scalar.activation(
                        out=et[:, 0:n, :],
                        in_=exp_ps[:]
                        .rearrange("p (s w) -> p s w", s=EXP_SLOTS)[:, 0:n, 0:Q],
                        func=Exp,
                        scale=float(DH) ** -0.5,
                    )
                    for s, (t, c, h2) in enumerate(group):
                        h = 2 * t + h2
                        if (t, h2) not in accs:
                            accs[(t, h2)] = aps.tile([33, Q], F32, tag="acc", name=f"acc_{b}_{t}_{h2}")
                        nc.tensor.matmul(
                            accs[(t, h2)][0:33, :],
                            vp[:, c, h * 33 : h * 33 + 33],
                            et[:, s, :],
                            start=(c == 0),
                            stop=(c == NC_KV - 1),
                        )
                    done += n
                    if pending_mm3 and done >= 4 * EXP_SLOTS:
                        pb, pattn = pending_mm3.pop()
                        emit_mm3(pb, pattn)
                    # finalize any pair whose last slot was in this group
                    for t, c, h2 in group:
                        if c == NC_KV - 1 and h2 == 1:
                            stages = []
                            for hh in range(2):
                                acc = accs.pop((t, hh))
                                # copy to SBUF right away to free the psum bank
                                stage = smalls.tile([33, Q], F32, tag="stage")
                                nc.vector.tensor_copy(out=stage[:], in_=acc[:])
                                stages.append(stage)
                            recips = []
                            for hh in range(2):
                                recip = smalls.tile([1, Q], F32, tag="recip")
                                nc.vector.reciprocal(
                                    out=recip[:], in_=stages[hh][32:33, :]
                                )
                                recips.append(recip)
                            rbs = []
                            for hh in range(2):
                                rb = smalls.tile([32, Q], F32, tag="rb")
                                nc.gpsimd.partition_broadcast(rb[:], recips[hh][:])
                                rbs.append(rb)
                            for hh in range(2):
                                h = 2 * t + hh
                                nc.vector.tensor_mul(
                                    out=attnT[
                                        (h % 4) * 32 : (h % 4) * 32 + 32, h // 4, :
                                    ],
                                    in0=stages[hh][0:32, :],
                                    in1=rbs[hh][:],
                                )

                # defer the output projection into the next batch's stream
                pending_mm3.append((b, attnT))

            while pending_mm3:
                pb, pattn = pending_mm3.pop()
                emit_mm3(pb, pattn)

    nc.compile()
    return nc


def _prep_core_inputs(hidden_states, key_states, value_states, W_out, b_out, c):
    lo, hi = c * BPC, (c + 1) * BPC
    hs = hidden_states[lo:hi]
    ks = key_states[lo:hi]
    vs = value_states[lo:hi]
    kt = np.ascontiguousarray(ks.transpose(0, 2, 1)).astype(NPBF16)
    ht = np.ascontiguousarray(hs.transpose(0, 2, 1)).astype(NPBF16)
    v4 = vs.reshape(BPC, NC_KV, 128, H, VD)
    vp = np.empty((BPC, NC_KV, 128, H, 33), dtype=NPBF16)
    vp[..., :VD] = v4
    vp[..., VD] = 1.0
    vp = vp.reshape(BPC, NC_KV, 128, H * 33)
    wt = np.ascontiguousarray(W_out.T).astype(NPBF16)
    bbt = np.broadcast_to(
        np.asarray(b_out, dtype=np.float32)[None, :], (128, O)
    ).copy()
    return {"KT": kt, "HT": ht, "VP": vp, "WT": wt, "BB": bbt}


def kernel(hidden_states, key_states, value_states, W_out, b_out):
    global _cached_nc
    hidden_states = np.asarray(hidden_states, dtype=np.float32)
    key_states = np.asarray(key_states, dtype=np.float32)
    value_states = np.asarray(value_states, dtype=np.float32)
    W_out = np.asarray(W_out, dtype=np.float32)
    b_out = np.asarray(b_out, dtype=np.float32)

    in_maps = [
        _prep_core_inputs(hidden_states, key_states, value_states, W_out, b_out, c)
        for c in range(N_CORES)
    ]
    if _cached_nc is None:
        _cached_nc = _build()
    res = run_bass_kernel_spmd(_cached_nc, in_maps, list(range(N_CORES)))
    return np.concatenate([r["OUT"] for r in res.results], axis=0)


# revision 10
# speedup vs baseline: 1.5730x; 1.0249x over previous
"""DabDetrAttention kernel for Trainium2, data-parallel over batch on 8 cores.

Strategy (per core, 4 batches):
  - Host-side prep: K^T / hidden^T (d-major) layouts, V chunks augmented with a
    ones column (softmax denominator rides along MM2 for free), W_out^T, bias
    broadcast. All matmuls run as fp32r (TF32-class, ~1.5e-4 rel err).
  - Per (batch, head): scores^T [kv, q] via K=64 matmuls row-packed two heads
    per PE pass; exp on ScalarE straight out of PSUM (3 score slots per
    instruction, scale=1/8 folded in); attn^T accumulated over 16 kv-chunks
    with M=33 matmuls (32 v-dims + ones row = denominator); normalize via
    reciprocal + gpsimd partition-broadcast + VectorE multiply; output
    projection as two accumulating matmuls per 128-row q-tile.
"""

import ml_dtypes
import numpy as np

import concourse.bacc as bacc
import concourse.tile as tile
from concourse import mybir
from concourse.bass_utils import run_bass_kernel_spmd

N_CORES = 8
B, Q, KV, D, H = 32, 300, 2048, 512, 8
DH, VD, O = 64, 32, 256
BPC = B // N_CORES          # batches per core
NC_KV = KV // 128           # kv chunks (16)
SLOT_W = 512                # psum bank width in fp32
EXP_SLOTS = 3               # score slots per exp instruction
QT = [(0, 128), (128, 128), (256, 44)]  # q tiles for the output projection

F32 = mybir.dt.float32
F32R = mybir.dt.float32r
BF16 = mybir.dt.bfloat16
NPBF16 = ml_dtypes.bfloat16
Exp = mybir.ActivationFunctionType.Exp

_cached_nc = None


def _build():
    nc = bacc.Bacc("TRN2", target_bir_lowering=False, debug=False)
    KT = nc.dram_tensor("KT", [BPC, D, KV], BF16, kind="ExternalInput").ap()
    HT = nc.dram_tensor("HT", [BPC, D, Q], BF16, kind="ExternalInput").ap()
    VP = nc.dram_tensor("VP", [BPC, NC_KV, 128, H * 33], BF16, kind="ExternalInput").ap()
    WT = nc.dram_tensor("WT", [O, O], BF16, kind="ExternalInput").ap()
    BB = nc.dram_tensor("BB", [128, O], F32, kind="ExternalInput").ap()
    OUT = nc.dram_tensor("OUT", [BPC, Q, O], F32, kind="ExternalOutput").ap()

    with tile.TileContext(nc) as tc:
        with (
            tc.tile_pool(name="consts", bufs=1) as consts,
            tc.tile_pool(name="ktp", bufs=2) as ktp,
            tc.tile_pool(name="htp", bufs=2) as htp,
            tc.tile_pool(name="vpp", bufs=2) as vpp,
            tc.tile_pool(name="etp", bufs=4) as etp,
            tc.tile_pool(name="attp", bufs=2) as attp,
            tc.tile_pool(name="smalls", bufs=8) as smalls,
            tc.tile_pool(name="outp", bufs=3) as outp,
            tc.tile_pool(name="eps", bufs=2, space="PSUM") as eps,
            tc.tile_pool(name="aps", bufs=2, space="PSUM") as aps,
        ):
            wt = consts.tile([128, 2, O], BF16, tag="wt")
            nc.sync.dma_start(
                out=wt[:], in_=WT.rearrange("(t p) o -> p t o", p=128)
            )
            bb = consts.tile([128, O], F32, tag="bb")
            nc.sync.dma_start(out=bb[:], in_=BB)

            def emit_loads(b):
                kt = ktp.tile([128, 4, KV], BF16, tag="kt", name=f"kt{b}")
                for t in range(4):
                    nc.sync.dma_start(
                        out=kt[:, t, :],
                        in_=KT[b].rearrange("(t p) k -> p t k", p=128)[:, t, :],
                    )
                ht = htp.tile([128, 4, Q], BF16, tag="ht", name=f"ht{b}")
                nc.sync.dma_start(
                    out=ht[:],
                    in_=HT[b].rearrange("(t p) q -> p t q", p=128),
                )
                vp = vpp.tile([128, NC_KV, H * 33], BF16, tag="vp", name=f"vp{b}")
                for half in range(2):
                    cs = half * (NC_KV // 2)
                    ce = cs + NC_KV // 2
                    nc.sync.dma_start(
                        out=vp[:, cs:ce, :],
                        in_=VP[b, cs:ce].rearrange("c p w -> p c w"),
                    )
                return kt, ht, vp

            loaded = {0: emit_loads(0)}
            pending_mm3 = []

            def emit_mm3(b, attnT):
                for q0, w in QT:
                    ps3 = aps.tile([128, SLOT_W], F32, tag="acc", name=f"mm3_{b}_{q0}")
                    for ci in range(2):
                        nc.tensor.matmul(
                            ps3[0:w, 0:O],
                            attnT[:, ci, q0 : q0 + w],
                            wt[:, ci, :],
                            start=(ci == 0),
                            stop=(ci == 1),
                        )
                    out_sb = outp.tile([128, O], F32, tag="out", name=f"o_{b}_{q0}")
                    nc.vector.tensor_add(
                        out=out_sb[0:w, :], in0=ps3[0:w, 0:O], in1=bb[0:w, :]
                    )
                    nc.gpsimd.dma_start(
                        out=OUT[b, q0 : q0 + w, :], in_=out_sb[0:w, :]
                    )

            for b in range(BPC):
                kt, ht, vp = loaded.pop(b)
                if b + 1 < BPC:
                    loaded[b + 1] = emit_loads(b + 1)
                attnT = attp.tile([128, 2, Q], BF16, tag="attnT")

                # flat slot stream: (pair t, chunk c, head-half h2)
                slots = [
                    (t, c, h2)
                    for t in range(4)
                    for c in range(NC_KV)
                    for h2 in range(2)
                ]
                accs = {}

                def emit_mm1_exp(group, n):
                    exp_ps = eps.tile(
                        [128, EXP_SLOTS * SLOT_W], F32, tag="exp", name="exp_ps"
                    )
                    for s, (t, c, h2) in enumerate(group):
                        lo = 64 * h2
                        nc.tensor.matmul(
                            exp_ps[:, s * SLOT_W : s * SLOT_W + Q].bitcast(F32),
                            kt[lo : lo + 64, t, c * 128 : (c + 1) * 128],
                            ht[lo : lo + 64, t, :],
                            start=True,
                            stop=True,
                            tile_position=(lo, 0),
                        )
                    et = etp.tile([128, EXP_SLOTS, Q], BF16, tag="et", name="et")
                    nc.scalar.activation(
                        out=et[:, 0:n, :],
                        in_=exp_ps[:]
                        .rearrange("p (s w) -> p s w", s=EXP_SLOTS)[:, 0:n, 0:Q],
                        func=Exp,
                        scale=float(DH) ** -0.5,
                    )
                    return et

                def emit_mm2(group, et):
                    for s, (t, c, h2) in enumerate(group):
                        h = 2 * t + h2
                        if (t, h2) not in accs:
                            accs[(t, h2)] = aps.tile(
                                [33, Q], F32, tag="acc", name=f"acc_{b}_{t}_{h2}"
                            )
                        nc.tensor.matmul(
                            accs[(t, h2)][0:33, :],
                            vp[:, c, h * 33 : h * 33 + 33],
                            et[:, s, :],
                            start=(c == 0),
                            stop=(c == NC_KV - 1),
                        )

                groups = []
                done = 0
                while done < len(slots):
                    groups.append(slots[done : done + EXP_SLOTS])
                    done += len(groups[-1])

                def finalize_pair(t):
                    stages = []
                    for hh in range(2):
                        acc = accs.pop((t, hh))
                        # copy to SBUF right away to free the psum bank
                        stage = smalls.tile([33, Q], F32, tag="stage", name="stg")
                        nc.vector.tensor_copy(out=stage[:], in_=acc[:])
                        stages.append(stage)
                    recips = []
                    for hh in range(2):
                        recip = smalls.tile([1, Q], F32, tag="recip", name="rc")
                        nc.vector.reciprocal(out=recip[:], in_=stages[hh][32:33, :])
                        recips.append(recip)
                    rbs = []
                    for hh in range(2):
                        rb = smalls.tile([32, Q], F32, tag="rb", name="rb")
                        nc.gpsimd.partition_broadcast(rb[:], recips[hh][:])
                        rbs.append(rb)
                    for hh in range(2):
                        h = 2 * t + hh
                        nc.vector.tensor_mul(
                            out=attnT[(h % 4) * 32 : (h % 4) * 32 + 32, h // 4, :],
                            in0=stages[hh][0:32, :],
                            in1=rbs[hh][:],
                        )

                def post_mm2(group):
                    for t, c, h2 in group:
                        if c == NC_KV - 1 and h2 == 1:
                            finalize_pair(t)

                prev = None  # (group, et) awaiting MM2
                done = 0
                for group in groups:
                    n = len(group)
                    et = emit_mm1_exp(group, n)
                    if prev is not None:
                        emit_mm2(*prev)
                        done += len(prev[0])
                        post_mm2(prev[0])
                    prev = (group, et)
                    if pending_mm3 and done >= 4 * EXP_SLOTS:
                        pb, pattn = pending_mm3.pop()
                        emit_mm3(pb, pattn)

                if prev is not None:
                    emit_mm2(*prev)
                    post_mm2(prev[0])

                # defer the output projection into the next batch's stream
                pending_mm3.append((b, attnT))

            while pending_mm3:
                pb, pattn = pending_mm3.pop()
                emit_mm3(pb, pattn)

    nc.compile()
    return nc


def _prep_core_inputs(hidden_states, key_states, value_states, W_out, b_out, c):
    lo, hi = c * BPC, (c + 1) * BPC
    hs = hidden_states[lo:hi]
    ks = key_states[lo:hi]
    vs = value_states[lo:hi]
    kt = np.ascontiguousarray(ks.transpose(0, 2, 1)).astype(NPBF16)
    ht = np.ascontiguousarray(hs.transpose(0, 2, 1)).astype(NPBF16)
    v4 = vs.reshape(BPC, NC_KV, 128, H, VD)
    vp = np.empty((BPC, NC_KV, 128, H, 33), dtype=NPBF16)
    vp[..., :VD] = v4
    vp[..., VD] = 1.0
    vp = vp.reshape(BPC, NC_KV, 128, H * 33)
    wt = np.ascontiguousarray(W_out.T).astype(NPBF16)
    bbt = np.broadcast_to(
        np.asarray(b_out, dtype=np.float32)[None, :], (128, O)
    ).copy()
    return {"KT": kt, "HT": ht, "VP": vp, "WT": wt, "BB": bbt}


def kernel(hidden_states, key_states, value_states, W_out, b_out):
    global _cached_nc
    hidden_states = np.asarray(hidden_states, dtype=np.float32)
    key_states = np.asarray(key_states, dtype=np.float32)
    value_states = np.asarray(value_states, dtype=np.float32)
    W_out = np.asarray(W_out, dtype=np.float32)
    b_out = np.asarray(b_out, dtype=np.float32)

    in_maps = [
        _prep_core_inputs(hidden_states, key_states, value_states, W_out, b_out, c)
        for c in range(N_CORES)
    ]
    if _cached_nc is None:
        _cached_nc = _build()
    res = run_bass_kernel_spmd(_cached_nc, in_maps, list(range(N_CORES)))
    return np.concatenate([r["OUT"] for r in res.results], axis=0)


# revision 11
# speedup vs baseline: 1.7094x; 1.0867x over previous
"""DabDetrAttention kernel for Trainium2, data-parallel over batch on 8 cores.

Strategy (per core, 4 batches):
  - Host-side prep: K^T / hidden^T (d-major) layouts, V chunks augmented with a
    ones column (softmax denominator rides along MM2 for free), W_out^T, bias
    broadcast. All matmuls run as fp32r (TF32-class, ~1.5e-4 rel err).
  - Per (batch, head): scores^T [kv, q] via K=64 matmuls row-packed two heads
    per PE pass; exp on ScalarE straight out of PSUM (3 score slots per
    instruction, scale=1/8 folded in); attn^T accumulated over 16 kv-chunks
    with M=33 matmuls (32 v-dims + ones row = denominator); normalize via
    reciprocal + gpsimd partition-broadcast + VectorE multiply; output
    projection as two accumulating matmuls per 128-row q-tile.
"""

import ml_dtypes
import numpy as np

import concourse.bacc as bacc
import concourse.tile as tile
from concourse import mybir
from concourse.bass_utils import run_bass_kernel_spmd

N_CORES = 8
B, Q, KV, D, H = 32, 300, 2048, 512, 8
DH, VD, O = 64, 32, 256
BPC = B // N_CORES          # batches per core
NC_KV = KV // 128           # kv chunks (16)
SLOT_W = 512                # psum bank width in fp32
EXP_SLOTS = 3               # score slots per exp instruction
QT = [(0, 128), (128, 128), (256, 44)]  # q tiles for the output projection

F32 = mybir.dt.float32
F32R = mybir.dt.float32r
BF16 = mybir.dt.bfloat16
NPBF16 = ml_dtypes.bfloat16
Exp = mybir.ActivationFunctionType.Exp

_cached_nc = None


def _build():
    nc = bacc.Bacc("TRN2", target_bir_lowering=False, debug=False)
    KT = nc.dram_tensor("KT", [BPC, D, KV], BF16, kind="ExternalInput").ap()
    HT = nc.dram_tensor("HT", [BPC, D, Q], BF16, kind="ExternalInput").ap()
    VP = nc.dram_tensor("VP", [BPC, NC_KV, 128, H * 33], BF16, kind="ExternalInput").ap()
    WT = nc.dram_tensor("WT", [O, O], BF16, kind="ExternalInput").ap()
    BB = nc.dram_tensor("BB", [128, O], F32, kind="ExternalInput").ap()
    OUT = nc.dram_tensor("OUT", [BPC, Q, O], F32, kind="ExternalOutput").ap()

    with tile.TileContext(nc) as tc:
        with (
            tc.tile_pool(name="consts", bufs=1) as consts,
            tc.tile_pool(name="ktp", bufs=2) as ktp,
            tc.tile_pool(name="htp", bufs=2) as htp,
            tc.tile_pool(name="vpp", bufs=2) as vpp,
            tc.tile_pool(name="etp", bufs=4) as etp,
            tc.tile_pool(name="attp", bufs=2) as attp,
            tc.tile_pool(name="smalls", bufs=8) as smalls,
            tc.tile_pool(name="outp", bufs=3) as outp,
            tc.tile_pool(name="eps", bufs=2, space="PSUM") as eps,
            tc.tile_pool(name="aps", bufs=2, space="PSUM") as aps,
        ):
            wt = consts.tile([128, 2, O], BF16, tag="wt")
            nc.sync.dma_start(
                out=wt[:], in_=WT.rearrange("(t p) o -> p t o", p=128)
            )
            bb = consts.tile([128, O], F32, tag="bb")
            nc.sync.dma_start(out=bb[:], in_=BB)

            def emit_loads(b):
                kt = ktp.tile([128, 4, KV], BF16, tag="kt", name=f"kt{b}")
                ht = htp.tile([128, 4, Q], BF16, tag="ht", name=f"ht{b}")
                vp = vpp.tile([128, NC_KV, H * 33], BF16, tag="vp", name=f"vp{b}")
                ktv = KT[b].rearrange("(t p) k -> p t k", p=128)
                nc.sync.dma_start(out=kt[:, 0, :], in_=ktv[:, 0, :])
                nc.sync.dma_start(
                    out=ht[:], in_=HT[b].rearrange("(t p) q -> p t q", p=128)
                )
                nq = NC_KV // 4
                nc.sync.dma_start(
                    out=vp[:, 0:nq, :], in_=VP[b, 0:nq].rearrange("c p w -> p c w")
                )
                for t in range(1, 4):
                    nc.sync.dma_start(out=kt[:, t, :], in_=ktv[:, t, :])
                for qtr in range(1, 4):
                    cs, ce = qtr * nq, (qtr + 1) * nq
                    nc.sync.dma_start(
                        out=vp[:, cs:ce, :],
                        in_=VP[b, cs:ce].rearrange("c p w -> p c w"),
                    )
                return kt, ht, vp

            loaded = {0: emit_loads(0)}
            pending_mm3 = []

            def emit_mm3(b, attnT):
                for q0, w in QT:
                    ps3 = aps.tile([128, SLOT_W], F32, tag="acc", name=f"mm3_{b}_{q0}")
                    for ci in range(2):
                        nc.tensor.matmul(
                            ps3[0:w, 0:O],
                            attnT[:, ci, q0 : q0 + w],
                            wt[:, ci, :],
                            start=(ci == 0),
                            stop=(ci == 1),
                        )
                    out_sb = outp.tile([128, O], F32, tag="out", name=f"o_{b}_{q0}")
                    nc.vector.tensor_add(
                        out=out_sb[0:w, :], in0=ps3[0:w, 0:O], in1=bb[0:w, :]
                    )
                    nc.gpsimd.dma_start(
                        out=OUT[b, q0 : q0 + w, :], in_=out_sb[0:w, :]
                    )

            for b in range(BPC):
                kt, ht, vp = loaded.pop(b)
                if b + 1 < BPC:
                    loaded[b + 1] = emit_loads(b + 1)
                attnT = attp.tile([128, 2, Q], BF16, tag="attnT")

                # flat slot stream: (pair t, chunk c, head-half h2)
                slots = [
                    (t, c, h2)
                    for t in range(4)
                    for c in range(NC_KV)
                    for h2 in range(2)
                ]
                accs = {}

                def emit_mm1_exp(group, n):
                    exp_ps = eps.tile(
                        [128, EXP_SLOTS * SLOT_W], F32, tag="exp", name="exp_ps"
                    )
                    for s, (t, c, h2) in enumerate(group):
                        lo = 64 * h2
                        nc.tensor.matmul(
                            exp_ps[:, s * SLOT_W : s * SLOT_W + Q].bitcast(F32),
                            kt[lo : lo + 64, t, c * 128 : (c + 1) * 128],
                            ht[lo : lo + 64, t, :],
                            start=True,
                            stop=True,
                            tile_position=(lo, 0),
                        )
                    et = etp.tile([128, EXP_SLOTS, Q], BF16, tag="et", name="et")
                    nc.scalar.activation(
                        out=et[:, 0:n, :],
                        in_=exp_ps[:]
                        .rearrange("p (s w) -> p s w", s=EXP_SLOTS)[:, 0:n, 0:Q],
                        func=Exp,
                        scale=float(DH) ** -0.5,
                    )
                    return et

                def emit_mm2(group, et):
                    for s, (t, c, h2) in enumerate(group):
                        h = 2 * t + h2
                        if (t, h2) not in accs:
                            accs[(t, h2)] = aps.tile(
                                [33, Q], F32, tag="acc", name=f"acc_{b}_{t}_{h2}"
                            )
                        nc.tensor.matmul(
                            accs[(t, h2)][0:33, :],
                            vp[:, c, h * 33 : h * 33 + 33],
                            et[:, s, :],
                            start=(c == 0),
                            stop=(c == NC_KV - 1),
                        )

                groups = []
                done = 0
                while done < len(slots):
                    groups.append(slots[done : done + EXP_SLOTS])
                    done += len(groups[-1])

                def finalize_pair(t):
                    stages = []
                    for hh in range(2):
                        acc = accs.pop((t, hh))
                        # copy to SBUF right away to free the psum bank
                        stage = smalls.tile([33, Q], F32, tag="stage", name="stg")
                        nc.vector.tensor_copy(out=stage[:], in_=acc[:])
                        stages.append(stage)
                    recips = []
                    for hh in range(2):
                        recip = smalls.tile([1, Q], F32, tag="recip", name="rc")
                        nc.vector.reciprocal_approx_fast(
                            out=recip[:], in_=stages[hh][32:33, :]
                        )
                        recips.append(recip)
                    rbs = []
                    for hh in range(2):
                        rb = smalls.tile([32, Q], F32, tag="rb", name="rb")
                        nc.gpsimd.partition_broadcast(rb[:], recips[hh][:])
                        rbs.append(rb)
                    for hh in range(2):
                        h = 2 * t + hh
                        nc.vector.tensor_mul(
                            out=attnT[(h % 4) * 32 : (h % 4) * 32 + 32, h // 4, :],
                            in0=stages[hh][0:32, :],
                            in1=rbs[hh][:],
                        )

                def post_mm2(group):
                    for t, c, h2 in group:
                        if c == NC_KV - 1 and h2 == 1:
                            finalize_pair(t)

                prev = None  # (group, et) awaiting MM2
                done = 0
                for group in groups:
                    n = len(group)
                    et = emit_mm1_exp(group, n)
                    if prev is not None:
                        emit_mm2(*prev)
                        done += len(prev[0])
                        post_mm2(prev[0])
                    prev = (group, et)
                    if pending_mm3 and done >= 4 * EXP_SLOTS:
                        pb, pattn = pending_mm3.pop()
                        emit_mm3(pb, pattn)

                if prev is not None:
                    emit_mm2(*prev)
                    post_mm2(prev[0])

                # defer the output projection into the next batch's stream
                pending_mm3.append((b, attnT))

            while pending_mm3:
                pb, pattn = pending_mm3.pop()
                emit_mm3(pb, pattn)

    nc.compile()
    return nc


def _prep_core_inputs(hidden_states, key_states, value_states, W_out, b_out, c):
    lo, hi = c * BPC, (c + 1) * BPC
    hs = hidden_states[lo:hi]
    ks = key_states[lo:hi]
    vs = value_states[lo:hi]
    kt = np.ascontiguousarray(ks.transpose(0, 2, 1)).astype(NPBF16)
    ht = np.ascontiguousarray(hs.transpose(0, 2, 1)).astype(NPBF16)
    v4 = vs.reshape(BPC, NC_KV, 128, H, VD)
    vp = np.empty((BPC, NC_KV, 128, H, 33), dtype=NPBF16)
    vp[..., :VD] = v4
    vp[..., VD] = 1.0
    vp = vp.reshape(BPC, NC_KV, 128, H * 33)
    wt = np.ascontiguousarray(W_out.T).astype(NPBF16)
    bbt = np.broadcast_to(
        np.asarray(b_out, dtype=np.float32)[None, :], (128, O)
    ).copy()
    return {"KT": kt, "HT": ht, "VP": vp, "WT": wt, "BB": bbt}


def kernel(hidden_states, key_states, value_states, W_out, b_out):
    global _cached_nc
    hidden_states = np.asarray(hidden_states, dtype=np.float32)
    key_states = np.asarray(key_states, dtype=np.float32)
    value_states = np.asarray(value_states, dtype=np.float32)
    W_out = np.asarray(W_out, dtype=np.float32)
    b_out = np.asarray(b_out, dtype=np.float32)

    in_maps = [
        _prep_core_inputs(hidden_states, key_states, value_states, W_out, b_out, c)
        for c in range(N_CORES)
    ]
    if _cached_nc is None:
        _cached_nc = _build()
    res = run_bass_kernel_spmd(_cached_nc, in_maps, list(range(N_CORES)))
    return np.concatenate([r["OUT"] for r in res.results], axis=0)
